# revision 17
# baseline (speedup 1.0000x reference)
"""DGN layer (gnn_message_passing) on 8 TRN2 NeuronCores.

Sharding: nodes split across 8 cores by destination range (graph parallel).
Host does index preprocessing + layout staging (edge sort/bucketing, padding
maps, dtype casts, mailbox-ordered staging of message rows); every float op
of the layer itself runs on device.

Per core, nodes are degree-sorted into 49 blocks of 128 dst.  For block b
the host stages the message stream in mailbox layout [128 dst, S_b slots,
128 feat] (feature innermost, replicate-last-edge padding, S_b multiple of
4; deg-0 rows are zeros), in BOTH bf16 (for DVE max/multiply) and fp8-e4m3
(for PE DoubleRow accumulation).  The device streams each block once per
dtype with affine DMAs at line rate, then:
  - sum_h: PE DoubleRow fp8 identity-matmuls accumulate 8 slot-planes per
    512-col matmul into a [128, 4*128] PSUM; DVE collapses the 4 planes.
    Replicate-padding corrected via -(S_b-deg)*msg_last.
  - w = |eig0_src - eig0_dst| per slot (padded slots stage
    eig0_src := eig0_dst so w == 0 exactly)
  - dir_num: DVE multiplies the bf16 stream by w (fp8 out), PE DoubleRow
    accumulates, DVE collapses.  den = tensor_reduce(add) of w.
  - max_h: DVE pairwise in-place max tree over slot slices (bf16).
  - mean/dir_av scaling on the Scalar engine, PE transposes -> lhsT tiles,
    3 matmuls against restacked W (BN scale folded), then snorm/BN
    shift/relu/residual.
"""

import math
import numpy as np

import ml_dtypes

import concourse.bass as bass
import concourse.bacc as bacc
import concourse.mybir as mybir
import concourse.tile as tile
from concourse.bass_utils import run_bass_kernel_spmd

F32 = mybir.dt.float32
BF16 = mybir.dt.bfloat16
FP8 = mybir.dt.float8e4
BF = ml_dtypes.bfloat16
F8 = ml_dtypes.float8_e4m3

AVG_D_LOG = float(np.log(33.0))
BN_EPS = 1e-5
D = 128
BLK = 128


class _Cfg:
    def __init__(self, n, e, n_cores):
        self.N = n
        self.E = e
        self.NC = n_cores
        assert n % n_cores == 0
        self.NPC = n // n_cores
        self.NBLK = math.ceil(self.NPC / BLK)
        self.NPC_PAD = self.NBLK * BLK


def _preprocess(cfg, h, eig, snorm_n, edge_src, edge_dst):
    """Index preprocessing + mailbox-layout staging."""
    N, NC, NPC = cfg.N, cfg.NC, cfg.NPC
    NPC_PAD, NBLK = cfg.NPC_PAD, cfg.NBLK

    deg_all = np.bincount(edge_dst, minlength=N).astype(np.int64)
    eorder = np.argsort(edge_dst, kind="stable")
    esrc_s = edge_src[eorder].astype(np.int64)
    row_start = np.zeros(N + 1, dtype=np.int64)
    np.cumsum(deg_all, out=row_start[1:])

    eig0_bf = np.ascontiguousarray(eig[:, 0]).astype(BF)
    # extended tables: row N is the zeros / 0.0 sentinel for empty mailboxes
    h_bf = h.astype(BF)
    h_ext = np.vstack([h_bf, np.zeros((1, D), dtype=BF)])
    h8_ext = h_ext.astype(F8)
    eig0_ext = np.concatenate([eig0_bf, np.zeros(1, dtype=BF)])

    # per-core degree-sorted node permutation (-1 = padding node)
    perms = []
    for c in range(NC):
        nodes = np.arange(c * NPC, (c + 1) * NPC, dtype=np.int64)
        p = nodes[np.argsort(-deg_all[nodes], kind="stable")]
        perm = np.full(NPC_PAD, -1, dtype=np.int64)
        perm[:NPC] = p
        perms.append(perm)
    perms = np.stack(perms)              # [NC, NPC_PAD]
    pdeg = np.where(perms >= 0, deg_all[np.clip(perms, 0, N - 1)], 0)

    # global (cross-core uniform) slots per block, multiple of 4 for the
    # PE 4-plane PSUM accumulation
    S_bs = [max(-4 * (-int(pdeg[:, b * BLK:(b + 1) * BLK].max()) // 4), 4)
            for b in range(NBLK)]
    SM_tot = sum(S_bs)
    moff = np.zeros(NBLK, dtype=np.int64)
    np.cumsum(S_bs[:-1], out=moff[1:])

    in_maps = []
    for c in range(NC):
        perm = perms[c]
        dg = pdeg[c]

        mstream = np.empty((128, SM_tot * D), dtype=BF)
        mstream8 = np.empty((128, SM_tot * D), dtype=F8)
        a_mail = np.empty((128, SM_tot), dtype=BF)
        bcol = np.zeros((128, NBLK), dtype=np.float32)
        negpad = np.zeros((128, NBLK), dtype=np.float32)

        for b in range(NBLK):
            S_b, off = S_bs[b], int(moff[b])
            g = perm[b * BLK:(b + 1) * BLK]              # [128] node ids
            k = dg[b * BLK:(b + 1) * BLK]                # [128] degrees
            gs = np.clip(g, 0, N - 1)
            # slot s -> edge row_start[g] + min(s, k-1); empty -> sentinel N
            slot = np.minimum(np.arange(S_b)[None, :],
                              np.maximum(k - 1, 0)[:, None])
            idx = row_start[gs][:, None] + slot
            src = np.where((k[:, None] > 0) & (g[:, None] >= 0),
                           esrc_s[np.minimum(idx, cfg.E - 1)], N)
            sl = slice(D * off, D * (off + S_b))
            mstream[:, sl] = h_ext[src].reshape(128, S_b * D)
            mstream8[:, sl] = h8_ext[src].reshape(128, S_b * D)
            bcol[:, b] = np.where(g >= 0, eig0_ext[gs], BF(0)).astype(np.float32)
            # padded slots: a := eig0_dst so w == 0 exactly
            a = np.where(np.arange(S_b)[None, :] < k[:, None],
                         eig0_ext[src], bcol[:, b:b + 1])
            a_mail[:, off:off + S_b] = a
            negpad[:, b] = -(S_b - k).astype(np.float32)

        degf = dg.astype(np.float32)
        deg_t = np.ascontiguousarray(degf.reshape(NBLK, BLK).T)
        safe = np.clip(perm, 0, N - 1)
        sn = np.where(perm >= 0, snorm_n[safe, 0], 0.0).astype(np.float32)
        snorm_t = np.ascontiguousarray(sn.reshape(NBLK, BLK).T)
        hin = np.where(perm[:, None] >= 0, h[safe], 0.0).astype(np.float32)

        in_maps.append(dict(
            mstream=mstream, mstream8=mstream8, a_mail=a_mail, bcol=bcol,
            negpad=negpad, deg_t=deg_t, snorm_t=snorm_t, hin=hin,
        ))

    meta = dict(perms=perms, S_bs=S_bs, moff=moff, SM_tot=SM_tot)
    return in_maps, meta


def _stage_consts(W, b, bn_gamma, bn_beta, bn_mean, bn_var):
    # W rows: c = i*384 + j*128 + f' (i = scale 0:id,1:amp,2:att;
    # j = agg 0:mean,1:max,2:dir).  wcat[:, j, i*128+f] = W[i*384+j*128+c, f]
    Wr = W.reshape(3, 3, 128, D)            # [i, j, c, f]
    wcat = np.ascontiguousarray(Wr.transpose(2, 1, 0, 3)).reshape(128, 3, 3 * D)
    bn = np.concatenate([bn_gamma, bn_beta, bn_mean, bn_var]).reshape(1, 4 * D)
    id8 = np.stack([np.eye(128, dtype=F8)] * 2, axis=1)   # [128, 2, 128]
    return dict(
        wcat=wcat.astype(np.float32),
        bvec=b.reshape(1, D).astype(np.float32),
        bn=bn.astype(np.float32),
        ident_bf=np.eye(128, dtype=BF),
        ident8=np.ascontiguousarray(id8.reshape(128, 2 * 128)),
    )


def _build_program(cfg, meta, has_bias):
    NBLK, NPC_PAD = cfg.NBLK, cfg.NPC_PAD
    S_bs, moff, SM_tot = meta["S_bs"], meta["moff"], meta["SM_tot"]
    Smax = max(S_bs)
    AOT = mybir.AluOpType
    AFT = mybir.ActivationFunctionType
    AXL = mybir.AxisListType
    MPM = mybir.MatmulPerfMode

    nc = bacc.Bacc("TRN2", target_bir_lowering=False, debug=False)

    mstream = nc.dram_tensor("mstream", [128, SM_tot * D], BF16,
                             kind="ExternalInput")
    mstream8 = nc.dram_tensor("mstream8", [128, SM_tot * D], FP8,
                              kind="ExternalInput")
    a_mail = nc.dram_tensor("a_mail", [128, SM_tot], BF16, kind="ExternalInput")
    bcol_d = nc.dram_tensor("bcol", [128, NBLK], F32, kind="ExternalInput")
    negpad_d = nc.dram_tensor("negpad", [128, NBLK], F32, kind="ExternalInput")
    deg_t = nc.dram_tensor("deg_t", [128, NBLK], F32, kind="ExternalInput")
    snorm_t = nc.dram_tensor("snorm_t", [128, NBLK], F32, kind="ExternalInput")
    hin = nc.dram_tensor("hin", [NPC_PAD, D], F32, kind="ExternalInput")
    wcat = nc.dram_tensor("wcat", [128, 3, 3 * D], F32, kind="ExternalInput")
    bvec = nc.dram_tensor("bvec", [1, D], F32, kind="ExternalInput")
    bn = nc.dram_tensor("bn", [1, 4 * D], F32, kind="ExternalInput")
    ident_bf_d = nc.dram_tensor("ident_bf", [128, 128], BF16,
                                kind="ExternalInput")
    ident8_d = nc.dram_tensor("ident8", [128, 2 * 128], FP8,
                              kind="ExternalInput")

    out_d = nc.dram_tensor("out", [NPC_PAD, D], F32, kind="ExternalOutput")

    with tile.TileContext(nc) as tc:
        with (
            tc.tile_pool(name="stage", bufs=1) as stg,
            tc.tile_pool(name="const", bufs=1) as cst,
            tc.tile_pool(name="mailp", bufs=3) as mailp,
            tc.tile_pool(name="mail8p", bufs=3) as mail8p,
            tc.tile_pool(name="tmp8p", bufs=3) as tmp8p,
            tc.tile_pool(name="wp", bufs=3) as wp,
            tc.tile_pool(name="red", bufs=3) as red,
            tc.tile_pool(name="agg", bufs=3) as agg,
            tc.tile_pool(name="ep", bufs=3) as ep,
            tc.tile_pool(name="psum_s", bufs=2, space="PSUM") as pss,
            tc.tile_pool(name="psum_d", bufs=2, space="PSUM") as psd,
            tc.tile_pool(name="ptp", bufs=2, space="PSUM") as ptp,
            tc.tile_pool(name="py", bufs=2, space="PSUM") as py,
        ):
            # ---------- staging loads ----------
            def load(dram, shape, dtype, pool=stg):
                t = pool.tile(shape, dtype, tag=dram.name)
                nc.sync.dma_start(t[:], dram[:])
                return t

            amail_s = load(a_mail, [128, SM_tot], BF16)
            bcol_s = load(bcol_d, [128, NBLK], F32)
            negpad_s = load(negpad_d, [128, NBLK], F32)
            degt_s = load(deg_t, [128, NBLK], F32)
            snormt_s = load(snorm_t, [128, NBLK], F32)
            bvec_s = load(bvec, [1, D], F32)
            bn_s = load(bn, [1, 4 * D], F32)
            identbf_s = load(ident_bf_d, [128, 128], BF16, pool=cst)
            ident8_s = load(ident8_d, [128, 2 * 128], FP8, pool=cst)
            wcat_s = load(wcat, [128, 3, 3 * D], F32)

            # ---------- bn fold / constant prep ----------
            g_r = bn_s[:, 0:D]
            beta_r = bn_s[:, D:2 * D]
            mean_r = bn_s[:, 2 * D:3 * D]
            var_r = bn_s[:, 3 * D:4 * D]
            bnsc = cst.tile([1, D], F32, tag="bnsc")
            eps_t = cst.tile([1, 1], F32, tag="eps_t")
            nc.gpsimd.memset(eps_t[:], BN_EPS)
            nc.scalar.activation(bnsc[:], var_r, AFT.Sqrt, bias=eps_t[:],
                                 scale=1.0)
            nc.vector.reciprocal(bnsc[:], bnsc[:])
            nc.vector.tensor_tensor(bnsc[:], bnsc[:], g_r, op=AOT.mult)
            shift = cst.tile([1, D], F32, tag="shift")       # beta - mean*scale
            nc.vector.tensor_tensor(shift[:], mean_r, bnsc[:], op=AOT.mult)
            nc.vector.tensor_tensor(shift[:], beta_r, shift[:], op=AOT.subtract)
            bprime = cst.tile([1, D], F32, tag="bprime")     # b * scale
            nc.vector.tensor_tensor(bprime[:], bvec_s[:], bnsc[:], op=AOT.mult)

            # broadcast const rows across partitions (DMA replicate via DRAM)
            rows_dram = nc.dram_tensor("cst_rows", [3, D], F32)
            nc.sync.dma_start(rows_dram[0:1, :], bnsc[:])
            nc.sync.dma_start(rows_dram[1:2, :], shift[:])
            nc.sync.dma_start(rows_dram[2:3, :], bprime[:])
            bnsc_bc = cst.tile([128, D], F32, tag="bnsc_bc")
            nc.sync.dma_start(bnsc_bc[:], rows_dram[0:1, :].to_broadcast([128, D]))
            shift_bc = cst.tile([128, D], F32, tag="shift_bc")
            nc.sync.dma_start(shift_bc[:], rows_dram[1:2, :].to_broadcast([128, D]))
            bprime_bc = cst.tile([128, D], F32, tag="bprime_bc")
            nc.sync.dma_start(bprime_bc[:], rows_dram[2:3, :].to_broadcast([128, D]))

            # wcat_bf = wcat * bn_scale -> bf16
            wcat_bf = cst.tile([128, 3, 3 * D], BF16, tag="wcatbf")
            nc.vector.tensor_tensor(
                wcat_bf[:].rearrange("p j (i d) -> p j i d", i=3),
                wcat_s[:].rearrange("p j (i d) -> p j i d", i=3),
                bnsc_bc[:, None, None, :].to_broadcast([128, 3, 3, D]),
                op=AOT.mult)

            # per-node scalar columns for ALL blocks at once
            invdeg_a = stg.tile([128, NBLK], F32, tag="invdeg_a")
            nc.vector.tensor_scalar(invdeg_a[:], degt_s[:], 1.0, None,
                                    op0=AOT.max)
            nc.vector.reciprocal(invdeg_a[:], invdeg_a[:])
            logd_a = stg.tile([128, NBLK], F32, tag="logd_a")
            nc.scalar.activation(logd_a[:], degt_s[:], AFT.Ln,
                                 bias=1.0, scale=1.0)
            amp_a = stg.tile([128, NBLK], F32, tag="amp_a")
            nc.vector.tensor_scalar(amp_a[:], logd_a[:], 1.0 / AVG_D_LOG,
                                    None, op0=AOT.mult)
            att_a = stg.tile([128, NBLK], F32, tag="att_a")
            nc.vector.tensor_scalar(att_a[:], logd_a[:], 1e-6, None,
                                    op0=AOT.max)
            nc.vector.reciprocal(att_a[:], att_a[:])
            nc.vector.tensor_scalar(att_a[:], att_a[:], AVG_D_LOG, None,
                                    op0=AOT.mult)

            id8_3 = ident8_s[:].rearrange("p (t x) -> p t x", t=2)

            for b in range(NBLK):
                S_b, mo = S_bs[b], int(moff[b])

                # ---- stream the block's mailbox (bf16 + fp8) ----
                mail = mailp.tile([128, Smax * D], BF16, tag="mail")
                nc.sync.dma_start(mail[:, 0:S_b * D],
                                  mstream[:, D * mo:D * (mo + S_b)])
                mail8 = mail8p.tile([128, Smax * D], FP8, tag="mail8")
                nc.sync.dma_start(mail8[:, 0:S_b * D],
                                  mstream8[:, D * mo:D * (mo + S_b)])
                m3 = mail[:, 0:S_b * D].rearrange("p (s f) -> p s f", f=D)

                # ---- w = |eig0_src - eig0_dst| per slot ----
                wt = wp.tile([128, Smax], BF16, tag="wt")
                nc.vector.tensor_scalar(wt[:, 0:S_b], amail_s[:, mo:mo + S_b],
                                        bcol_s[:, b:b + 1], None,
                                        op0=AOT.subtract)
                nc.scalar.activation(wt[:, 0:S_b], wt[:, 0:S_b], AFT.Abs)
                den = wp.tile([128, 1], F32, tag="den")
                nc.vector.tensor_reduce(den[:], wt[:, 0:S_b], axis=AXL.X,
                                        op=AOT.add)
                nc.vector.tensor_scalar(den[:], den[:], 1e-30, None,
                                        op0=AOT.add)
                nc.vector.reciprocal(den[:], den[:])

                # ---- sum via PE DoubleRow fp8 ----
                n8 = S_b // 8
                tail = (S_b % 8) == 4
                nmm = n8 + (1 if tail else 0)
                ps_sum = pss.tile([128, 4 * D], F32, tag="ps_sum")
                for g in range(n8):
                    nc.tensor.matmul(
                        ps_sum[:], id8_3,
                        mail8[:, g * 8 * D:(g + 1) * 8 * D]
                            .rearrange("p (t x) -> p t x", t=2),
                        start=(g == 0), stop=(not tail and g == n8 - 1),
                        perf_mode=MPM.DoubleRow, skip_group_check=True)
                if tail:
                    nc.tensor.matmul(
                        ps_sum[:], ident8_s[:, 0:128],
                        mail8[:, n8 * 8 * D:(n8 * 8 + 4) * D],
                        start=(n8 == 0), stop=True, skip_group_check=True)
                sum_t = red.tile([128, D], F32, tag="sum")
                nc.vector.tensor_reduce(
                    sum_t[:], ps_sum[:].rearrange("p (s f) -> p f s", s=4),
                    axis=AXL.X, op=AOT.add)
                # pad correction: padding replicates the LAST edge's message
                nc.vector.scalar_tensor_tensor(
                    sum_t[:], mail8[:, (S_b - 1) * D:S_b * D],
                    negpad_s[:, b:b + 1], sum_t[:],
                    op0=AOT.mult, op1=AOT.add)

                # ---- dir: multiply by expanded w, PE accumulate ----
                ng4 = S_b // 4
                w4 = wp.tile([128, 4 * Smax], BF16, tag="w4")
                nc.vector.tensor_copy(
                    w4[:, 0:4 * S_b].rearrange("p (s j) -> p s j", j=4),
                    wt[:, 0:S_b, None].to_broadcast([128, S_b, 4]))
                tmp16 = tmp8p.tile([128, Smax * D], BF16, tag="tmp16")
                nc.vector.tensor_tensor(
                    tmp16[:, 0:S_b * D]
                        .rearrange("p (s f2 f1) -> p s f2 f1", f2=32, f1=4),
                    mail[:, 0:S_b * D]
                        .rearrange("p (s f2 f1) -> p s f2 f1", f2=32, f1=4),
                    w4[:, 0:4 * S_b]
                        .rearrange("p (s j) -> p s j", j=4)[:, :, None, :]
                        .to_broadcast([128, S_b, 32, 4]),
                    op=AOT.mult)
                ps_dir = psd.tile([128, 4 * D], F32, tag="ps_dir")
                for g in range(ng4):
                    nc.tensor.matmul(
                        ps_dir[:], identbf_s[:],
                        tmp16[:, g * 4 * D:(g + 1) * 4 * D],
                        start=(g == 0), stop=(g == ng4 - 1),
                        skip_group_check=True)
                dir_t = red.tile([128, D], F32, tag="dir")
                nc.vector.tensor_reduce(
                    dir_t[:], ps_dir[:].rearrange("p (s f) -> p f s", s=4),
                    axis=AXL.X, op=AOT.add)

                # ---- max via in-place pairwise tree on the bf16 stream ----
                n = S_b
                while n > 2:
                    hh = (n + 1) // 2
                    nc.vector.tensor_tensor(
                        m3[:, 0:hh, :], m3[:, 0:hh, :],
                        m3[:, n - hh:n, :], op=AOT.max)
                    n = hh
                mx_c = agg.tile([128, D], BF16, tag="mx_c")
                nc.vector.tensor_tensor(
                    mx_c[:, None, :], m3[:, 0:1, :], m3[:, 1:2, :],
                    op=AOT.max)
                mx_t = mx_c[:]                          # [128, D] bf16

                # ---- scale to mean / dir_av (bf16) on Scalar engine ----
                mean_bf = agg.tile([128, D], BF16, tag="mean_bf")
                nc.scalar.activation(mean_bf[:], sum_t[:], AFT.Copy,
                                     scale=invdeg_a[:, b:b + 1])
                dir_bf = agg.tile([128, D], BF16, tag="dir_bf")
                nc.scalar.activation(dir_bf[:], dir_t[:], AFT.Copy,
                                     scale=den[:])

                # ---- transpose aggregates -> lhsT [feat, dst] ----
                lhs = []
                for src_t in (mean_bf[:], mx_t, dir_bf[:]):
                    tp = ptp.tile([128, 128], BF16, tag="tp")
                    nc.tensor.transpose(tp[:], src_t, identbf_s[:])
                    l_t = agg.tile([128, 128], BF16, tag="lhs")
                    nc.scalar.copy(l_t[:], tp[:])
                    lhs.append(l_t)

                # ---- final matmuls + combine ----
                y_ps = py.tile([128, 3 * D], F32, tag="y")
                for j, l_t in enumerate(lhs):
                    nc.tensor.matmul(y_ps[:], l_t[:], wcat_bf[:, j, :],
                                     start=(j == 0), stop=(j == 2))

                y1_sb = ep.tile([128, D], F32, tag="y1_sb")
                nc.scalar.copy(y1_sb[:], y_ps[:, 0:D])
                u = ep.tile([128, D], F32, tag="u")
                nc.vector.scalar_tensor_tensor(
                    u[:], y_ps[:, D:2 * D], amp_a[:, b:b + 1], y1_sb[:],
                    op0=AOT.mult, op1=AOT.add)
                v = ep.tile([128, D], F32, tag="v")
                nc.vector.scalar_tensor_tensor(
                    v[:], y_ps[:, 2 * D:3 * D], att_a[:, b:b + 1], u[:],
                    op0=AOT.mult, op1=AOT.add)
                if has_bias:
                    nc.vector.tensor_tensor(v[:], v[:], bprime_bc[:],
                                            op=AOT.add)
                nc.vector.scalar_tensor_tensor(
                    v[:], v[:], snormt_s[:, b:b + 1], shift_bc[:],
                    op0=AOT.mult, op1=AOT.add)
                hin_t = ep.tile([128, D], F32, tag="hin")
                nc.sync.dma_start(hin_t[:], hin[b * BLK:(b + 1) * BLK, :])
                out_t = ep.tile([128, D], F32, tag="out")
                nc.vector.scalar_tensor_tensor(
                    out_t[:], v[:], 0.0, hin_t[:], op0=AOT.max, op1=AOT.add)
                nc.sync.dma_start(out_d[b * BLK:(b + 1) * BLK, :], out_t[:])

    nc.compile()
    return nc


_CACHE = {}


def _run(h, eig, snorm_n, W, b, bn_gamma, bn_beta, bn_mean, bn_var,
         edge_src, edge_dst, n_cores=8, trace=False, sim=False):
    N, E = h.shape[0], edge_src.shape[0]
    cfg = _Cfg(N, E, n_cores)
    in_maps, meta = _preprocess(cfg, h, eig, snorm_n, edge_src, edge_dst)
    consts = _stage_consts(W, b, bn_gamma, bn_beta, bn_mean, bn_var)
    for m in in_maps:
        m.update(consts)
    has_bias = bool(np.any(b != 0))

    key = (N, E, n_cores, has_bias, tuple(meta["S_bs"]))
    if key not in _CACHE:
        _CACHE[key] = _build_program(cfg, meta, has_bias)
    nc = _CACHE[key]

    if sim:
        from concourse.bass_interp import CoreSim
        csim = CoreSim(nc)
        for k, v in in_maps[0].items():
            csim.tensor(k)[:] = v
        csim.simulate()
        results = [{"out": np.array(csim.tensor("out"))}]
        n_out = 1
        res = None
    else:
        res = run_bass_kernel_spmd(nc, in_maps, core_ids=list(range(n_cores)),
                                   trace=trace)
        results = res.results
        n_out = n_cores

    out = np.empty((N, D), dtype=np.float32)
    for c in range(n_out):
        perm = meta["perms"][c]
        oc = results[c]["out"]
        valid = perm >= 0
        out[perm[valid]] = oc[valid]
    return out, res


def kernel(**inputs):
    out, _ = _run(
        np.asarray(inputs["h"]), np.asarray(inputs["eig"]),
        np.asarray(inputs["snorm_n"]), np.asarray(inputs["W"]),
        np.asarray(inputs["b"]), np.asarray(inputs["bn_gamma"]),
        np.asarray(inputs["bn_beta"]), np.asarray(inputs["bn_mean"]),
        np.asarray(inputs["bn_var"]), np.asarray(inputs["edge_src"]),
        np.asarray(inputs["edge_dst"]))
    return out


# revision 19
# speedup vs baseline: 1.1838x; 1.1838x over previous
"""DGN layer (gnn_message_passing) on 8 TRN2 NeuronCores.

Sharding: nodes split across 8 cores by destination range (graph parallel).
Host does index preprocessing + layout staging (edge sort/bucketing, padding
maps, dtype casts, mailbox-ordered staging of message rows); every float op
of the layer itself runs on device.

Per core, nodes are degree-sorted into 49 blocks of 128 dst.  For block b
the host stages the message stream in mailbox layout [128 dst, S_b slots,
128 feat] (feature innermost, replicate-last-edge padding, S_b multiple of
4; deg-0 rows are zeros), in BOTH bf16 (for DVE max/multiply) and fp8-e4m3
(for PE DoubleRow accumulation).  The device streams each block once per
dtype with affine DMAs at line rate, then:
  - sum_h: PE DoubleRow fp8 identity-matmuls accumulate 8 slot-planes per
    512-col matmul into a [128, 4*128] PSUM; DVE collapses the 4 planes.
    Replicate-padding corrected via -(S_b-deg)*msg_last.
  - w = |eig0_src - eig0_dst| per slot (padded slots stage
    eig0_src := eig0_dst so w == 0 exactly)
  - dir_num: DVE multiplies the bf16 stream by w (w expanded 4x on device
    so the broadcast has a real-stride innermost run), plain PE
    identity-matmuls accumulate 4 slot-planes per 512-col matmul, DVE
    collapses.  den = tensor_reduce(add) of w.
  - max_h: DVE pairwise in-place max tree over slot slices (bf16).
  - mean/dir_av scaling on the Scalar engine, PE transposes -> lhsT tiles,
    3 matmuls against restacked W (BN scale folded), then snorm/BN
    shift/relu/residual.
"""

import math
import numpy as np

import ml_dtypes

import concourse.bass as bass
import concourse.bacc as bacc
import concourse.mybir as mybir
import concourse.tile as tile
from concourse.bass_utils import run_bass_kernel_spmd

F32 = mybir.dt.float32
BF16 = mybir.dt.bfloat16
FP8 = mybir.dt.float8e4
BF = ml_dtypes.bfloat16
F8 = ml_dtypes.float8_e4m3

AVG_D_LOG = float(np.log(33.0))
BN_EPS = 1e-5
D = 128
BLK = 128


class _Cfg:
    def __init__(self, n, e, n_cores):
        self.N = n
        self.E = e
        self.NC = n_cores
        assert n % n_cores == 0
        self.NPC = n // n_cores
        self.NBLK = math.ceil(self.NPC / BLK)
        self.NPC_PAD = self.NBLK * BLK


def _preprocess(cfg, h, eig, snorm_n, edge_src, edge_dst):
    """Index preprocessing + mailbox-layout staging."""
    N, NC, NPC = cfg.N, cfg.NC, cfg.NPC
    NPC_PAD, NBLK = cfg.NPC_PAD, cfg.NBLK

    deg_all = np.bincount(edge_dst, minlength=N).astype(np.int64)
    eorder = np.argsort(edge_dst, kind="stable")
    esrc_s = edge_src[eorder].astype(np.int64)
    row_start = np.zeros(N + 1, dtype=np.int64)
    np.cumsum(deg_all, out=row_start[1:])

    eig0_bf = np.ascontiguousarray(eig[:, 0]).astype(BF)
    # extended tables: row N is the zeros / 0.0 sentinel for empty mailboxes
    h_bf = h.astype(BF)
    h_ext = np.vstack([h_bf, np.zeros((1, D), dtype=BF)])
    h8_ext = h_ext.astype(F8)
    eig0_ext = np.concatenate([eig0_bf, np.zeros(1, dtype=BF)])

    # per-core degree-sorted node permutation (-1 = padding node)
    perms = []
    for c in range(NC):
        nodes = np.arange(c * NPC, (c + 1) * NPC, dtype=np.int64)
        p = nodes[np.argsort(-deg_all[nodes], kind="stable")]
        perm = np.full(NPC_PAD, -1, dtype=np.int64)
        perm[:NPC] = p
        perms.append(perm)
    perms = np.stack(perms)              # [NC, NPC_PAD]
    pdeg = np.where(perms >= 0, deg_all[np.clip(perms, 0, N - 1)], 0)

    # global (cross-core uniform) slots per block, multiple of 4 for the
    # PE 4-plane PSUM accumulation
    S_bs = [max(-4 * (-int(pdeg[:, b * BLK:(b + 1) * BLK].max()) // 4), 4)
            for b in range(NBLK)]
    SM_tot = sum(S_bs)
    moff = np.zeros(NBLK, dtype=np.int64)
    np.cumsum(S_bs[:-1], out=moff[1:])

    in_maps = []
    for c in range(NC):
        perm = perms[c]
        dg = pdeg[c]

        mstream = np.empty((128, SM_tot * D), dtype=BF)
        mstream8 = np.empty((128, SM_tot * D), dtype=F8)
        a_mail = np.empty((128, SM_tot), dtype=BF)
        bcol = np.zeros((128, NBLK), dtype=np.float32)
        negpad = np.zeros((128, NBLK), dtype=np.float32)

        for b in range(NBLK):
            S_b, off = S_bs[b], int(moff[b])
            g = perm[b * BLK:(b + 1) * BLK]              # [128] node ids
            k = dg[b * BLK:(b + 1) * BLK]                # [128] degrees
            gs = np.clip(g, 0, N - 1)
            # slot s -> edge row_start[g] + min(s, k-1); empty -> sentinel N
            slot = np.minimum(np.arange(S_b)[None, :],
                              np.maximum(k - 1, 0)[:, None])
            idx = row_start[gs][:, None] + slot
            src = np.where((k[:, None] > 0) & (g[:, None] >= 0),
                           esrc_s[np.minimum(idx, cfg.E - 1)], N)
            sl = slice(D * off, D * (off + S_b))
            mstream[:, sl] = h_ext[src].reshape(128, S_b * D)
            mstream8[:, sl] = h8_ext[src].reshape(128, S_b * D)
            bcol[:, b] = np.where(g >= 0, eig0_ext[gs], BF(0)).astype(np.float32)
            # padded slots: a := eig0_dst so w == 0 exactly
            a = np.where(np.arange(S_b)[None, :] < k[:, None],
                         eig0_ext[src], bcol[:, b:b + 1])
            a_mail[:, off:off + S_b] = a
            negpad[:, b] = -(S_b - k).astype(np.float32)

        degf = dg.astype(np.float32)
        deg_t = np.ascontiguousarray(degf.reshape(NBLK, BLK).T)
        safe = np.clip(perm, 0, N - 1)
        sn = np.where(perm >= 0, snorm_n[safe, 0], 0.0).astype(np.float32)
        snorm_t = np.ascontiguousarray(sn.reshape(NBLK, BLK).T)
        hin = np.where(perm[:, None] >= 0, h[safe], 0.0).astype(np.float32)

        in_maps.append(dict(
            mstream=mstream, mstream8=mstream8, a_mail=a_mail, bcol=bcol,
            negpad=negpad, deg_t=deg_t, snorm_t=snorm_t, hin=hin,
        ))

    meta = dict(perms=perms, S_bs=S_bs, moff=moff, SM_tot=SM_tot)
    return in_maps, meta


def _stage_consts(W, b, bn_gamma, bn_beta, bn_mean, bn_var):
    # W rows: c = i*384 + j*128 + f' (i = scale 0:id,1:amp,2:att;
    # j = agg 0:mean,1:max,2:dir).  wcat[:, j, i*128+f] = W[i*384+j*128+c, f]
    Wr = W.reshape(3, 3, 128, D)            # [i, j, c, f]
    wcat = np.ascontiguousarray(Wr.transpose(2, 1, 0, 3)).reshape(128, 3, 3 * D)
    bn = np.concatenate([bn_gamma, bn_beta, bn_mean, bn_var]).reshape(1, 4 * D)
    id8 = np.stack([np.eye(128, dtype=F8)] * 2, axis=1)   # [128, 2, 128]
    return dict(
        wcat=wcat.astype(np.float32),
        bvec=b.reshape(1, D).astype(np.float32),
        bn=bn.astype(np.float32),
        ident_bf=np.eye(128, dtype=BF),
        ident8=np.ascontiguousarray(id8.reshape(128, 2 * 128)),
    )


def _build_program(cfg, meta, has_bias):
    NBLK, NPC_PAD = cfg.NBLK, cfg.NPC_PAD
    S_bs, moff, SM_tot = meta["S_bs"], meta["moff"], meta["SM_tot"]
    Smax = max(S_bs)
    AOT = mybir.AluOpType
    AFT = mybir.ActivationFunctionType
    AXL = mybir.AxisListType
    MPM = mybir.MatmulPerfMode

    nc = bacc.Bacc("TRN2", target_bir_lowering=False, debug=False)

    mstream = nc.dram_tensor("mstream", [128, SM_tot * D], BF16,
                             kind="ExternalInput")
    mstream8 = nc.dram_tensor("mstream8", [128, SM_tot * D], FP8,
                              kind="ExternalInput")
    a_mail = nc.dram_tensor("a_mail", [128, SM_tot], BF16, kind="ExternalInput")
    bcol_d = nc.dram_tensor("bcol", [128, NBLK], F32, kind="ExternalInput")
    negpad_d = nc.dram_tensor("negpad", [128, NBLK], F32, kind="ExternalInput")
    deg_t = nc.dram_tensor("deg_t", [128, NBLK], F32, kind="ExternalInput")
    snorm_t = nc.dram_tensor("snorm_t", [128, NBLK], F32, kind="ExternalInput")
    hin = nc.dram_tensor("hin", [NPC_PAD, D], F32, kind="ExternalInput")
    wcat = nc.dram_tensor("wcat", [128, 3, 3 * D], F32, kind="ExternalInput")
    bvec = nc.dram_tensor("bvec", [1, D], F32, kind="ExternalInput")
    bn = nc.dram_tensor("bn", [1, 4 * D], F32, kind="ExternalInput")
    ident_bf_d = nc.dram_tensor("ident_bf", [128, 128], BF16,
                                kind="ExternalInput")
    ident8_d = nc.dram_tensor("ident8", [128, 2 * 128], FP8,
                              kind="ExternalInput")

    out_d = nc.dram_tensor("out", [NPC_PAD, D], F32, kind="ExternalOutput")

    with tile.TileContext(nc) as tc:
        with (
            tc.tile_pool(name="stage", bufs=1) as stg,
            tc.tile_pool(name="const", bufs=1) as cst,
            tc.tile_pool(name="mailp", bufs=2) as mailp,
            tc.tile_pool(name="mail8p", bufs=2) as mail8p,
            tc.tile_pool(name="tmp8p", bufs=2) as tmp8p,
            tc.tile_pool(name="wp", bufs=2) as wp,
            tc.tile_pool(name="red", bufs=2) as red,
            tc.tile_pool(name="agg", bufs=2) as agg,
            tc.tile_pool(name="ep", bufs=2) as ep,
            tc.tile_pool(name="psum_s", bufs=2, space="PSUM") as pss,
            tc.tile_pool(name="psum_d", bufs=2, space="PSUM") as psd,
            tc.tile_pool(name="ptp", bufs=2, space="PSUM") as ptp,
            tc.tile_pool(name="py", bufs=2, space="PSUM") as py,
        ):
            # ---------- staging loads ----------
            def load(dram, shape, dtype, pool=stg):
                t = pool.tile(shape, dtype, tag=dram.name)
                nc.sync.dma_start(t[:], dram[:])
                return t

            amail_s = load(a_mail, [128, SM_tot], BF16)
            bcol_s = load(bcol_d, [128, NBLK], F32)
            negpad_s = load(negpad_d, [128, NBLK], F32)
            degt_s = load(deg_t, [128, NBLK], F32)
            snormt_s = load(snorm_t, [128, NBLK], F32)
            bvec_s = load(bvec, [1, D], F32)
            bn_s = load(bn, [1, 4 * D], F32)
            identbf_s = load(ident_bf_d, [128, 128], BF16, pool=cst)
            ident8_s = load(ident8_d, [128, 2 * 128], FP8, pool=cst)
            wcat_s = load(wcat, [128, 3, 3 * D], F32)

            # ---------- bn fold / constant prep ----------
            g_r = bn_s[:, 0:D]
            beta_r = bn_s[:, D:2 * D]
            mean_r = bn_s[:, 2 * D:3 * D]
            var_r = bn_s[:, 3 * D:4 * D]
            bnsc = cst.tile([1, D], F32, tag="bnsc")
            eps_t = cst.tile([1, 1], F32, tag="eps_t")
            nc.gpsimd.memset(eps_t[:], BN_EPS)
            nc.scalar.activation(bnsc[:], var_r, AFT.Sqrt, bias=eps_t[:],
                                 scale=1.0)
            nc.vector.reciprocal(bnsc[:], bnsc[:])
            nc.vector.tensor_tensor(bnsc[:], bnsc[:], g_r, op=AOT.mult)
            shift = cst.tile([1, D], F32, tag="shift")       # beta - mean*scale
            nc.vector.tensor_tensor(shift[:], mean_r, bnsc[:], op=AOT.mult)
            nc.vector.tensor_tensor(shift[:], beta_r, shift[:], op=AOT.subtract)
            bprime = cst.tile([1, D], F32, tag="bprime")     # b * scale
            nc.vector.tensor_tensor(bprime[:], bvec_s[:], bnsc[:], op=AOT.mult)

            # broadcast const rows across partitions (DMA replicate via DRAM)
            rows_dram = nc.dram_tensor("cst_rows", [3, D], F32)
            nc.sync.dma_start(rows_dram[0:1, :], bnsc[:])
            nc.sync.dma_start(rows_dram[1:2, :], shift[:])
            nc.sync.dma_start(rows_dram[2:3, :], bprime[:])
            bnsc_bc = cst.tile([128, D], F32, tag="bnsc_bc")
            nc.sync.dma_start(bnsc_bc[:], rows_dram[0:1, :].to_broadcast([128, D]))
            shift_bc = cst.tile([128, D], F32, tag="shift_bc")
            nc.sync.dma_start(shift_bc[:], rows_dram[1:2, :].to_broadcast([128, D]))
            bprime_bc = cst.tile([128, D], F32, tag="bprime_bc")
            nc.sync.dma_start(bprime_bc[:], rows_dram[2:3, :].to_broadcast([128, D]))

            # wcat_bf = wcat * bn_scale -> bf16
            wcat_bf = cst.tile([128, 3, 3 * D], BF16, tag="wcatbf")
            nc.vector.tensor_tensor(
                wcat_bf[:].rearrange("p j (i d) -> p j i d", i=3),
                wcat_s[:].rearrange("p j (i d) -> p j i d", i=3),
                bnsc_bc[:, None, None, :].to_broadcast([128, 3, 3, D]),
                op=AOT.mult)

            # per-node scalar columns for ALL blocks at once
            invdeg_a = stg.tile([128, NBLK], F32, tag="invdeg_a")
            nc.vector.tensor_scalar(invdeg_a[:], degt_s[:], 1.0, None,
                                    op0=AOT.max)
            nc.vector.reciprocal(invdeg_a[:], invdeg_a[:])
            logd_a = stg.tile([128, NBLK], F32, tag="logd_a")
            nc.scalar.activation(logd_a[:], degt_s[:], AFT.Ln,
                                 bias=1.0, scale=1.0)
            amp_a = stg.tile([128, NBLK], F32, tag="amp_a")
            nc.vector.tensor_scalar(amp_a[:], logd_a[:], 1.0 / AVG_D_LOG,
                                    None, op0=AOT.mult)
            att_a = stg.tile([128, NBLK], F32, tag="att_a")
            nc.vector.tensor_scalar(att_a[:], logd_a[:], 1e-6, None,
                                    op0=AOT.max)
            nc.vector.reciprocal(att_a[:], att_a[:])
            nc.vector.tensor_scalar(att_a[:], att_a[:], AVG_D_LOG, None,
                                    op0=AOT.mult)

            id8_3 = ident8_s[:].rearrange("p (t x) -> p t x", t=2)

            for b in range(NBLK):
                S_b, mo = S_bs[b], int(moff[b])

                # ---- stream the block's mailbox (bf16 + fp8) ----
                mail = mailp.tile([128, Smax * D], BF16, tag="mail")
                nc.sync.dma_start(mail[:, 0:S_b * D],
                                  mstream[:, D * mo:D * (mo + S_b)])
                mail8 = mail8p.tile([128, Smax * D], FP8, tag="mail8")
                nc.sync.dma_start(mail8[:, 0:S_b * D],
                                  mstream8[:, D * mo:D * (mo + S_b)])
                m3 = mail[:, 0:S_b * D].rearrange("p (s f) -> p s f", f=D)

                # ---- w = |eig0_src - eig0_dst| per slot ----
                wt = wp.tile([128, Smax], BF16, tag="wt")
                nc.vector.tensor_scalar(wt[:, 0:S_b], amail_s[:, mo:mo + S_b],
                                        bcol_s[:, b:b + 1], None,
                                        op0=AOT.subtract)
                nc.scalar.activation(wt[:, 0:S_b], wt[:, 0:S_b], AFT.Abs)
                den = wp.tile([128, 1], F32, tag="den")
                nc.vector.tensor_reduce(den[:], wt[:, 0:S_b], axis=AXL.X,
                                        op=AOT.add)
                nc.vector.tensor_scalar(den[:], den[:], 1e-30, None,
                                        op0=AOT.add)
                nc.vector.reciprocal(den[:], den[:])

                # ---- sum via PE DoubleRow fp8 ----
                n8 = S_b // 8
                tail = (S_b % 8) == 4
                nmm = n8 + (1 if tail else 0)
                ps_sum = pss.tile([128, 4 * D], F32, tag="ps_sum")
                for g in range(n8):
                    nc.tensor.matmul(
                        ps_sum[:], id8_3,
                        mail8[:, g * 8 * D:(g + 1) * 8 * D]
                            .rearrange("p (t x) -> p t x", t=2),
                        start=(g == 0), stop=(not tail and g == n8 - 1),
                        perf_mode=MPM.DoubleRow, skip_group_check=True)
                if tail:
                    nc.tensor.matmul(
                        ps_sum[:], ident8_s[:, 0:128],
                        mail8[:, n8 * 8 * D:(n8 * 8 + 4) * D],
                        start=(n8 == 0), stop=True, skip_group_check=True)
                sum_t = red.tile([128, D], F32, tag="sum")
                nc.vector.tensor_reduce(
                    sum_t[:], ps_sum[:].rearrange("p (s f) -> p f s", s=4),
                    axis=AXL.X, op=AOT.add)
                # pad correction: padding replicates the LAST edge's message
                nc.vector.scalar_tensor_tensor(
                    sum_t[:], mail8[:, (S_b - 1) * D:S_b * D],
                    negpad_s[:, b:b + 1], sum_t[:],
                    op0=AOT.mult, op1=AOT.add)

                # ---- dir: multiply by expanded w, PE accumulate ----
                ng4 = S_b // 4
                w4 = wp.tile([128, 4 * Smax], BF16, tag="w4")
                nc.vector.tensor_copy(
                    w4[:, 0:4 * S_b].rearrange("p (s j) -> p s j", j=4),
                    wt[:, 0:S_b, None].to_broadcast([128, S_b, 4]))
                tmp16 = tmp8p.tile([128, Smax * D], BF16, tag="tmp16")
                nc.vector.tensor_tensor(
                    tmp16[:, 0:S_b * D]
                        .rearrange("p (s f2 f1) -> p s f2 f1", f2=32, f1=4),
                    mail[:, 0:S_b * D]
                        .rearrange("p (s f2 f1) -> p s f2 f1", f2=32, f1=4),
                    w4[:, 0:4 * S_b]
                        .rearrange("p (s j) -> p s j", j=4)[:, :, None, :]
                        .to_broadcast([128, S_b, 32, 4]),
                    op=AOT.mult)
                ps_dir = psd.tile([128, 4 * D], F32, tag="ps_dir")
                for g in range(ng4):
                    nc.tensor.matmul(
                        ps_dir[:], identbf_s[:],
                        tmp16[:, g * 4 * D:(g + 1) * 4 * D],
                        start=(g == 0), stop=(g == ng4 - 1),
                        skip_group_check=True)
                dir_t = red.tile([128, D], F32, tag="dir")
                nc.vector.tensor_reduce(
                    dir_t[:], ps_dir[:].rearrange("p (s f) -> p f s", s=4),
                    axis=AXL.X, op=AOT.add)

                # ---- max via in-place pairwise tree on the bf16 stream ----
                n = S_b
                while n > 2:
                    hh = (n + 1) // 2
                    nc.vector.tensor_tensor(
                        m3[:, 0:hh, :], m3[:, 0:hh, :],
                        m3[:, n - hh:n, :], op=AOT.max)
                    n = hh
                mx_c = agg.tile([128, D], BF16, tag="mx_c")
                nc.vector.tensor_tensor(
                    mx_c[:, None, :], m3[:, 0:1, :], m3[:, 1:2, :],
                    op=AOT.max)
                mx_t = mx_c[:]                          # [128, D] bf16

                # ---- scale to mean / dir_av (bf16) on Scalar engine ----
                mean_bf = agg.tile([128, D], BF16, tag="mean_bf")
                nc.scalar.activation(mean_bf[:], sum_t[:], AFT.Copy,
                                     scale=invdeg_a[:, b:b + 1])
                dir_bf = agg.tile([128, D], BF16, tag="dir_bf")
                nc.scalar.activation(dir_bf[:], dir_t[:], AFT.Copy,
                                     scale=den[:])

                # ---- transpose aggregates -> lhsT [feat, dst] ----
                lhs = []
                for src_t in (mean_bf[:], mx_t, dir_bf[:]):
                    tp = ptp.tile([128, 128], BF16, tag="tp")
                    nc.tensor.transpose(tp[:], src_t, identbf_s[:])
                    l_t = agg.tile([128, 128], BF16, tag="lhs")
                    nc.scalar.copy(l_t[:], tp[:])
                    lhs.append(l_t)

                # ---- final matmuls + combine ----
                y_ps = py.tile([128, 3 * D], F32, tag="y")
                for j, l_t in enumerate(lhs):
                    nc.tensor.matmul(y_ps[:], l_t[:], wcat_bf[:, j, :],
                                     start=(j == 0), stop=(j == 2))

                y1_sb = ep.tile([128, D], F32, tag="y1_sb")
                nc.scalar.copy(y1_sb[:], y_ps[:, 0:D])
                u = ep.tile([128, D], F32, tag="u")
                nc.vector.scalar_tensor_tensor(
                    u[:], y_ps[:, D:2 * D], amp_a[:, b:b + 1], y1_sb[:],
                    op0=AOT.mult, op1=AOT.add)
                v = ep.tile([128, D], F32, tag="v")
                nc.vector.scalar_tensor_tensor(
                    v[:], y_ps[:, 2 * D:3 * D], att_a[:, b:b + 1], u[:],
                    op0=AOT.mult, op1=AOT.add)
                if has_bias:
                    nc.vector.tensor_tensor(v[:], v[:], bprime_bc[:],
                                            op=AOT.add)
                nc.vector.scalar_tensor_tensor(
                    v[:], v[:], snormt_s[:, b:b + 1], shift_bc[:],
                    op0=AOT.mult, op1=AOT.add)
                hin_t = ep.tile([128, D], F32, tag="hin")
                nc.sync.dma_start(hin_t[:], hin[b * BLK:(b + 1) * BLK, :])
                out_t = ep.tile([128, D], F32, tag="out")
                nc.vector.scalar_tensor_tensor(
                    out_t[:], v[:], 0.0, hin_t[:], op0=AOT.max, op1=AOT.add)
                nc.sync.dma_start(out_d[b * BLK:(b + 1) * BLK, :], out_t[:])

    nc.compile()
    return nc


_CACHE = {}


def _run(h, eig, snorm_n, W, b, bn_gamma, bn_beta, bn_mean, bn_var,
         edge_src, edge_dst, n_cores=8, trace=False, sim=False):
    N, E = h.shape[0], edge_src.shape[0]
    cfg = _Cfg(N, E, n_cores)
    in_maps, meta = _preprocess(cfg, h, eig, snorm_n, edge_src, edge_dst)
    consts = _stage_consts(W, b, bn_gamma, bn_beta, bn_mean, bn_var)
    for m in in_maps:
        m.update(consts)
    has_bias = bool(np.any(b != 0))

    key = (N, E, n_cores, has_bias, tuple(meta["S_bs"]))
    if key not in _CACHE:
        _CACHE[key] = _build_program(cfg, meta, has_bias)
    nc = _CACHE[key]

    if sim:
        from concourse.bass_interp import CoreSim
        csim = CoreSim(nc)
        for k, v in in_maps[0].items():
            csim.tensor(k)[:] = v
        csim.simulate()
        results = [{"out": np.array(csim.tensor("out"))}]
        n_out = 1
        res = None
    else:
        res = run_bass_kernel_spmd(nc, in_maps, core_ids=list(range(n_cores)),
                                   trace=trace)
        results = res.results
        n_out = n_cores

    out = np.empty((N, D), dtype=np.float32)
    for c in range(n_out):
        perm = meta["perms"][c]
        oc = results[c]["out"]
        valid = perm >= 0
        out[perm[valid]] = oc[valid]
    return out, res


def kernel(**inputs):
    out, _ = _run(
        np.asarray(inputs["h"]), np.asarray(inputs["eig"]),
        np.asarray(inputs["snorm_n"]), np.asarray(inputs["W"]),
        np.asarray(inputs["b"]), np.asarray(inputs["bn_gamma"]),
        np.asarray(inputs["bn_beta"]), np.asarray(inputs["bn_mean"]),
        np.asarray(inputs["bn_var"]), np.asarray(inputs["edge_src"]),
        np.asarray(inputs["edge_dst"]))
    return out
